# revision 14
# baseline (speedup 1.0000x reference)
"""CrossAttention kernel for 8 trn2 NeuronCores (v6).

Reference:
  q = x @ Wq          [n, vq, h]
  k = y @ Wk          [n, vk, h]
  v = y @ Wv          [n, vk, c]
  out = softmax(q k^T / sqrt(h)) @ v        [n, vq, c]
with N=4, VQ=VK=4096, C=128, H=64, fp32.

Sharding: 8 cores = 4 batches x 2 query halves.

v6 (v1 154us, v2 135us, v3 103us, v4 112us, v5 130us):
  - fine-grained 512-col staging: tiny DMA->proj->cast chains so the first
    exp fires as soon as ~128KB have landed; critical descriptors first,
    staging spread across the two hwdge queues (sync: yT, scalar: wqk/xT/y).
  - bf16 everywhere on the PE streams (q/k/y/attn); exp input stays fp32
    PSUM, z accumulation fp32.
  - per chunk (2 vk tiles): PE = 2 score + 2 z streams (~1030ns), ScalarE =
    one exp [128,1024] (~1150ns, the bottleneck), VectorE = bf16 partial
    sums. Per-j folded sums parked in SBUF; all softmax-sum matmuls +
    normalization constants handled in the tail / on host.
  - PSUM: scores pool 3x2 banks (also used by proj bounces), z 2 banks.
    PE warm-up matmuls hide the HAM ramp during the first DMAs.
"""

import sys

sys.path.insert(0, "/opt/trn_rl_repo")

from contextlib import ExitStack

import ml_dtypes
import numpy as np

import concourse.bass as bass
import concourse.tile as tile
from concourse import mybir
from concourse.bass_utils import run_bass_kernel_spmd

F32 = mybir.dt.float32
F32R = mybir.dt.float32r
BF16 = mybir.dt.bfloat16
P = 128

N, VQ, VK, C, H = 4, 4096, 4096, 128, 64
VQ_PER = VQ // 2          # 2048 queries per core
SCALE = float(H) ** -0.5

VQ_T = 512                # vq tile (psum free dim)
N_VQ_T = VQ_PER // VQ_T   # 4
N_VK_T = VK // P          # 32 vk tiles of 128
CHUNK = 2                 # vk tiles per exp chunk
N_CH = N_VK_T // CHUNK    # 16 chunks per vq tile
ST = 512                  # staging granularity (columns)


def _split_multi_waits(nc):
    """walrus in this env supports one sync-wait per instruction; hoist
    extras onto same-engine NoOps inserted just before."""
    for fn in nc.m.functions:
        for bb in fn.blocks:
            out = []
            for inst in bb.instructions:
                si = inst.sync_info
                waits = list(si.on_wait) if si and si.on_wait else []
                if len(waits) > 1:
                    for w in waits[:-1]:
                        out.append(mybir.InstNoOp(
                            name=nc.get_next_instruction_name(),
                            engine=inst.engine,
                            ins=[], outs=[],
                            sync_info=mybir.SyncInfo(on_wait=[w], on_update=[]),
                        ))
                    inst.sync_info = mybir.SyncInfo(
                        on_wait=[waits[-1]],
                        on_update=list(si.on_update) if si.on_update else [],
                    )
                out.append(inst)
            bb.instructions = out


def _build():
    nc = bass.Bass()
    xT_d = nc.declare_dram_parameter("xT", [C, VQ_PER], BF16, isOutput=False)
    yT_d = nc.declare_dram_parameter("yT", [C, VK], BF16, isOutput=False)
    y_d = nc.declare_dram_parameter("y", [P, N_VK_T * P], BF16, isOutput=False)
    wqk_d = nc.declare_dram_parameter("wqk", [C, 2 * H], BF16, isOutput=False)
    wv_d = nc.declare_dram_parameter("wv", [C, C], F32R, isOutput=False)
    oT_d = nc.declare_dram_parameter("oT", [C, VQ_PER], F32, isOutput=True)
    sums_d = nc.declare_dram_parameter("sums", [1, VQ_PER], F32, isOutput=True)

    with tile.TileContext(nc) as tc, ExitStack() as ctx:
        const = ctx.enter_context(tc.tile_pool(name="const", bufs=1))
        persist = ctx.enter_context(tc.tile_pool(name="persist", bufs=1))

        # ---- constants (no DMA deps) ----
        wqk_sb = const.tile([P, 2 * H], BF16)
        wv_sb = const.tile([P, C], F32R)
        ones_f = const.tile([P, 1], F32)
        nc.vector.memset(ones_f[:], 1.0)
        ones_b = const.tile([P, 1], BF16)
        nc.vector.tensor_copy(ones_b[:], ones_f[:])
        junk_b = const.tile([P, VQ_T], BF16)
        nc.vector.memset(junk_b[:], 0.0)

        # ---- persistent tensors ----
        qT = persist.tile([64, VQ_PER], BF16)
        kT = persist.tile([64, VK], BF16)
        y_sb = persist.tile([P, N_VK_T, P], BF16)     # y tiles [vk, c]
        attn = persist.tile([P, N_VK_T * VQ_T], BF16)  # per vq tile, rotating
        acc = persist.tile([P, CHUNK * VQ_T], BF16)   # DVE partial sums
        acc2 = persist.tile([P, VQ_PER], BF16)        # folded sums, per j
        z_sb = persist.tile([P, VQ_PER], F32R)        # z = y^T attn  [c, vq]
        oT_sb = persist.tile([P, VQ_PER], F32)        # Wv^T z        [c, vq]
        srow = persist.tile([1, VQ_PER], F32)         # softmax sums

        with ExitStack() as mctx:
            # sc pool also hosts the projection bounces (smaller tiles)
            sc_ps = mctx.enter_context(
                tc.tile_pool(name="sc_ps", bufs=3, space="PSUM"))
            z_ps = mctx.enter_context(
                tc.tile_pool(name="z_ps", bufs=2, space="PSUM"))

            stage = mctx.enter_context(tc.tile_pool(name="stage", bufs=1))
            qx_stage = [stage.tile([P, ST], BF16, name=f"qx{i}")
                        for i in range(VQ_PER // ST)]
            ky_stage = [stage.tile([P, ST], BF16, name=f"ky{i}")
                        for i in range(VK // ST)]

            # ---- staging: DMA + projections (emitted interleaved below) --
            def dma_wqk():
                nc.scalar.dma_start(wqk_sb[:], wqk_d[:])

            def dma_wv():
                nc.scalar.dma_start(wv_sb[:], wv_d[:])

            def dma_x(s):  # s 0..3, 512 cols of xT
                nc.scalar.dma_start(
                    qx_stage[s][:], xT_d[:, s * ST:(s + 1) * ST])

            def dma_yT(s):  # s 0..7, 512 vk cols of yT
                nc.sync.dma_start(
                    ky_stage[s][:], yT_d[:, s * ST:(s + 1) * ST])

            def dma_y_raw(q):  # quarter q 0..3: 8 vk tiles, partition-major
                nc.scalar.dma_start(
                    y_sb[:, 8 * q:8 * (q + 1), :].rearrange("p t c -> p (t c)"),
                    y_d[:, 8 * q * P:8 * (q + 1) * P])

            def proj_x(s):
                ps = sc_ps.tile([64, ST], F32, tag="sc", name=f"pjx{s}")
                nc.tensor.matmul(ps[:], wqk_sb[:, 0:H], qx_stage[s][:],
                                 start=True, stop=True)
                nc.vector.tensor_copy(qT[:, s * ST:(s + 1) * ST], ps[:])

            def proj_y(s):
                ps = sc_ps.tile([64, ST], F32, tag="sc", name=f"pjy{s}")
                nc.tensor.matmul(ps[:], wqk_sb[:, H:], ky_stage[s][:],
                                 start=True, stop=True)
                nc.vector.tensor_copy(kT[:, s * ST:(s + 1) * ST], ps[:])

            def warmup():
                # keep the PE HAM-warm while the first DMAs land
                wu = z_ps.tile([P, VQ_T], F32, tag="z", name="wu")
                for _ in range(14):
                    nc.tensor.matmul(wu[0:1, :], ones_b[:], junk_b[:],
                                     start=True, stop=True)

            # ---- flash loop ----
            z_tiles = [None] * N_VQ_T

            def emit_scores_exp(j, c):
                sc = sc_ps.tile([P, CHUNK * VQ_T], F32, tag="sc")
                s = CHUNK * c
                for ii in range(CHUNK):
                    nc.tensor.matmul(
                        sc[:, ii * VQ_T:(ii + 1) * VQ_T],
                        kT[:, (s + ii) * P:(s + ii + 1) * P],
                        qT[:, j * VQ_T:(j + 1) * VQ_T],
                        start=True, stop=True)
                nc.scalar.activation(
                    attn[:, s * VQ_T:(s + 2) * VQ_T],
                    sc[:],
                    mybir.ActivationFunctionType.Exp, scale=SCALE)

            def emit_consume(j, c):
                if c == 0:
                    zp = z_ps.tile([P, VQ_T], F32, tag="z", name=f"z{j}")
                    z_tiles[j] = zp
                zp = z_tiles[j]
                for ii in range(CHUNK):
                    i = CHUNK * c + ii
                    a_sl = attn[:, i * VQ_T:(i + 1) * VQ_T]
                    nc.tensor.matmul(
                        zp[:], y_sb[:, i, :], a_sl,
                        start=(i == 0), stop=(i == N_VK_T - 1))
                # VectorE partial-sum accumulation (both tiles in one op)
                ch_sl = attn[:, CHUNK * c * VQ_T:CHUNK * (c + 1) * VQ_T]
                if c == 0:
                    nc.vector.tensor_copy(acc[:], ch_sl)
                else:
                    nc.vector.tensor_tensor(
                        out=acc[:], in0=acc[:], in1=ch_sl,
                        op=mybir.AluOpType.add)
                if c == N_CH - 1:
                    jsl = slice(j * VQ_T, (j + 1) * VQ_T)
                    nc.vector.tensor_tensor(
                        out=acc2[:, jsl], in0=acc[:, 0:VQ_T],
                        in1=acc[:, VQ_T:], op=mybir.AluOpType.add)
                    nc.vector.tensor_copy(z_sb[:, jsl], zp[:])

            # staging schedule: pre-work, then 1 bundle per early flash chunk
            pre = [lambda: dma_yT(0), dma_wqk, lambda: dma_x(0),
                   lambda: dma_y_raw(0), warmup,
                   lambda: proj_y(0), lambda: proj_x(0)]
            bg = [
                lambda: dma_yT(1),
                lambda: proj_y(1),
                lambda: (dma_yT(2), dma_y_raw(1)),
                lambda: proj_y(2),
                lambda: (dma_yT(3), dma_x(1)),
                lambda: proj_y(3),
                lambda: (dma_yT(4), dma_y_raw(2)),
                lambda: proj_y(4),
                lambda: (dma_yT(5), dma_x(2)),
                lambda: proj_y(5),
                lambda: (dma_yT(6), dma_y_raw(3)),
                lambda: proj_y(6),
                lambda: (dma_yT(7), dma_x(3), dma_wv()),
                lambda: proj_y(7),
                lambda: proj_x(1),
                lambda: proj_x(2),
                lambda: proj_x(3),
            ]
            for t in pre:
                t()

            work = [(j, c) for j in range(N_VQ_T) for c in range(N_CH)]
            for n, (j, c) in enumerate(work):
                emit_scores_exp(j, c)
                if n < len(bg):
                    bg[n]()
                if n > 0:
                    emit_consume(*work[n - 1])
            emit_consume(*work[-1])

        # ---- tail: softmax sums + oT = Wv^T z, store ----
        with ExitStack() as fctx:
            f_ps = fctx.enter_context(
                tc.tile_pool(name="f_ps", bufs=2, space="PSUM"))
            s_ps = fctx.enter_context(
                tc.tile_pool(name="s_ps", bufs=2, space="PSUM"))
            for j in range(N_VQ_T):
                sl = slice(j * VQ_T, (j + 1) * VQ_T)
                sm = s_ps.tile([1, VQ_T], F32, tag="sm")
                nc.tensor.matmul(sm[:], ones_b[:], acc2[:, sl],
                                 start=True, stop=True)
                nc.scalar.copy(srow[:, sl], sm[:])
                o2 = f_ps.tile([P, VQ_T], F32, tag="o2")
                nc.tensor.matmul(o2[:], wv_sb[:], z_sb[:, sl],
                                 start=True, stop=True)
                nc.vector.tensor_copy(oT_sb[:, sl], o2[:])
                nc.sync.dma_start(oT_d[:, sl], oT_sb[:, sl])
            nc.sync.dma_start(sums_d[:], srow[:])

    _split_multi_waits(nc)
    return nc


_NC = None


def _get_nc():
    global _NC
    if _NC is None:
        _NC = _build()
    return _NC


def make_in_maps(x, y, Wq, Wk, Wv):
    bf = ml_dtypes.bfloat16
    x = np.ascontiguousarray(x, dtype=np.float32)
    y = np.ascontiguousarray(y, dtype=np.float32)
    wqk = np.ascontiguousarray(
        np.concatenate([Wq, Wk], axis=1), dtype=np.float32).astype(bf)
    wv = np.ascontiguousarray(Wv, dtype=np.float32)
    in_maps = []
    for core in range(8):
        n, half = core // 2, core % 2
        yb = y[n].astype(bf)
        # pre-tiled, partition-major: [p, t, c] for vk row = 128*t + p
        y_tiled = np.ascontiguousarray(
            yb.reshape(N_VK_T, P, C).transpose(1, 0, 2).reshape(P, -1))
        in_maps.append({
            "xT": np.ascontiguousarray(
                x[n, half * VQ_PER:(half + 1) * VQ_PER, :].T).astype(bf),
            "yT": np.ascontiguousarray(y[n].T).astype(bf),
            "y": y_tiled,
            "wqk": wqk, "wv": wv,
        })
    return in_maps


def finish(results):
    """Host-side epilogue: normalize + transpose per core shard."""
    out = np.empty((N, VQ, C), dtype=np.float32)
    for core in range(8):
        n, half = core // 2, core % 2
        r = results[core]
        out[n, half * VQ_PER:(half + 1) * VQ_PER, :] = (
            r["oT"] / r["sums"]).T
    return out


def kernel(x, y, Wq, Wk, Wv):
    nc = _get_nc()
    in_maps = make_in_maps(x, y, Wq, Wk, Wv)
    res = run_bass_kernel_spmd(nc, in_maps, list(range(8)))
    return finish(res.results)


# revision 19
# speedup vs baseline: 1.0024x; 1.0024x over previous
"""CrossAttention kernel for 8 trn2 NeuronCores (v6).

Reference:
  q = x @ Wq          [n, vq, h]
  k = y @ Wk          [n, vk, h]
  v = y @ Wv          [n, vk, c]
  out = softmax(q k^T / sqrt(h)) @ v        [n, vq, c]
with N=4, VQ=VK=4096, C=128, H=64, fp32.

Sharding: 8 cores = 4 batches x 2 query halves.

v6 (v1 154us, v2 135us, v3 103us, v4 112us, v5 130us):
  - fine-grained 512-col staging: tiny DMA->proj->cast chains so the first
    exp fires as soon as ~128KB have landed; critical descriptors first,
    staging spread across the two hwdge queues (sync: yT, scalar: wqk/xT/y).
  - bf16 everywhere on the PE streams (q/k/y/attn); exp input stays fp32
    PSUM, z accumulation fp32.
  - per chunk (2 vk tiles): PE = 2 score + 2 z streams (~1030ns), ScalarE =
    one exp [128,1024] (~1150ns, the bottleneck), VectorE = bf16 partial
    sums. Per-j folded sums parked in SBUF; all softmax-sum matmuls +
    normalization constants handled in the tail / on host.
  - PSUM: scores pool 3x2 banks (also used by proj bounces), z 2 banks.
    PE warm-up matmuls hide the HAM ramp during the first DMAs.
"""

import sys

sys.path.insert(0, "/opt/trn_rl_repo")

from contextlib import ExitStack

import ml_dtypes
import numpy as np

import concourse.bass as bass
import concourse.tile as tile
from concourse import mybir
from concourse.bass_utils import run_bass_kernel_spmd

F32 = mybir.dt.float32
F32R = mybir.dt.float32r
BF16 = mybir.dt.bfloat16
P = 128

N, VQ, VK, C, H = 4, 4096, 4096, 128, 64
VQ_PER = VQ // 2          # 2048 queries per core
SCALE = float(H) ** -0.5

VQ_T = 512                # vq tile (psum free dim)
N_VQ_T = VQ_PER // VQ_T   # 4
N_VK_T = VK // P          # 32 vk tiles of 128
CHUNK = 2                 # vk tiles per exp chunk
N_CH = N_VK_T // CHUNK    # 16 chunks per vq tile
ST = 512                  # staging granularity (columns)


def _split_multi_waits(nc):
    """walrus in this env supports one sync-wait per instruction; hoist
    extras onto same-engine NoOps inserted just before."""
    for fn in nc.m.functions:
        for bb in fn.blocks:
            out = []
            for inst in bb.instructions:
                si = inst.sync_info
                waits = list(si.on_wait) if si and si.on_wait else []
                if len(waits) > 1:
                    for w in waits[:-1]:
                        out.append(mybir.InstNoOp(
                            name=nc.get_next_instruction_name(),
                            engine=inst.engine,
                            ins=[], outs=[],
                            sync_info=mybir.SyncInfo(on_wait=[w], on_update=[]),
                        ))
                    inst.sync_info = mybir.SyncInfo(
                        on_wait=[waits[-1]],
                        on_update=list(si.on_update) if si.on_update else [],
                    )
                out.append(inst)
            bb.instructions = out


def _build():
    nc = bass.Bass()
    xT_d = nc.declare_dram_parameter("xT", [C, VQ_PER], BF16, isOutput=False)
    yT_d = nc.declare_dram_parameter("yT", [C, VK], BF16, isOutput=False)
    y_d = nc.declare_dram_parameter("y", [P, N_VK_T * P], BF16, isOutput=False)
    wqk_d = nc.declare_dram_parameter("wqk", [C, 2 * H], BF16, isOutput=False)
    wv_d = nc.declare_dram_parameter("wv", [C, C], F32R, isOutput=False)
    oT_d = nc.declare_dram_parameter("oT", [C, VQ_PER], F32, isOutput=True)
    sums_d = nc.declare_dram_parameter("sums", [1, VQ_PER], F32, isOutput=True)

    with tile.TileContext(nc) as tc, ExitStack() as ctx:
        const = ctx.enter_context(tc.tile_pool(name="const", bufs=1))
        persist = ctx.enter_context(tc.tile_pool(name="persist", bufs=1))

        # ---- constants (no DMA deps) ----
        wqk_sb = const.tile([P, 2 * H], BF16)
        wv_sb = const.tile([P, C], F32R)
        ones_f = const.tile([P, 1], F32)
        nc.vector.memset(ones_f[:], 1.0)
        ones_b = const.tile([P, 1], BF16)
        nc.vector.tensor_copy(ones_b[:], ones_f[:])
        junk_b = const.tile([P, VQ_T], BF16)
        nc.vector.memset(junk_b[:], 0.0)

        # ---- persistent tensors ----
        qT = persist.tile([64, VQ_PER], BF16)
        kT = persist.tile([64, VK], BF16)
        y_sb = persist.tile([P, N_VK_T, P], BF16)     # y tiles [vk, c]
        attn = persist.tile([P, N_VK_T * VQ_T], BF16)  # per vq tile, rotating
        acc = persist.tile([P, CHUNK * VQ_T], BF16)   # DVE partial sums
        acc2 = persist.tile([P, VQ_PER], BF16)        # folded sums, per j
        z_sb = persist.tile([P, VQ_PER], F32R)        # z = y^T attn  [c, vq]
        oT_sb = persist.tile([P, VQ_PER], F32)        # Wv^T z        [c, vq]
        srow = persist.tile([1, VQ_PER], F32)         # softmax sums

        with ExitStack() as mctx:
            # sc pool also hosts the projection bounces (smaller tiles)
            sc_ps = mctx.enter_context(
                tc.tile_pool(name="sc_ps", bufs=3, space="PSUM"))
            z_ps = mctx.enter_context(
                tc.tile_pool(name="z_ps", bufs=2, space="PSUM"))

            stage = mctx.enter_context(tc.tile_pool(name="stage", bufs=1))
            qx_stage = stage.tile([P, VQ_PER], BF16)
            ky_stage = stage.tile([P, VK], BF16)

            # ---- staging: DMA + projections (emitted interleaved below) --
            def dma_wqk():
                nc.scalar.dma_start(wqk_sb[:], wqk_d[:])

            def dma_wv():
                nc.scalar.dma_start(wv_sb[:], wv_d[:])

            def dma_x_all():  # xT in one contiguous DMA (512KB bf16)
                nc.sync.dma_start(qx_stage[:], xT_d[:])

            def dma_yT(s):  # s 0..7, 512 vk cols of yT
                nc.sync.dma_start(
                    ky_stage[:, s * ST:(s + 1) * ST],
                    yT_d[:, s * ST:(s + 1) * ST])

            def dma_y_raw(q):  # quarter q 0..3: 8 vk tiles, partition-major
                nc.scalar.dma_start(
                    y_sb[:, 8 * q:8 * (q + 1), :].rearrange("p t c -> p (t c)"),
                    y_d[:, 8 * q * P:8 * (q + 1) * P])

            def proj_x(s):
                sl = slice(s * ST, (s + 1) * ST)
                ps = sc_ps.tile([64, ST], F32, tag="sc", name=f"pjx{s}")
                nc.tensor.matmul(ps[:], wqk_sb[:, 0:H], qx_stage[:, sl],
                                 start=True, stop=True)
                nc.vector.tensor_copy(qT[:, sl], ps[:])

            def proj_y(s):
                sl = slice(s * ST, (s + 1) * ST)
                ps = sc_ps.tile([64, ST], F32, tag="sc", name=f"pjy{s}")
                nc.tensor.matmul(ps[:], wqk_sb[:, H:], ky_stage[:, sl],
                                 start=True, stop=True)
                nc.vector.tensor_copy(kT[:, sl], ps[:])

            def warmup():
                # keep the PE HAM-warm while the first DMAs land
                wu = z_ps.tile([P, VQ_T], F32, tag="z", name="wu")
                for _ in range(14):
                    nc.tensor.matmul(wu[0:1, :], ones_b[:], junk_b[:],
                                     start=True, stop=True)

            # ---- flash loop ----
            z_tiles = [None] * N_VQ_T

            def emit_scores_exp(j, c):
                sc = sc_ps.tile([P, CHUNK * VQ_T], F32, tag="sc")
                s = CHUNK * c
                for ii in range(CHUNK):
                    nc.tensor.matmul(
                        sc[:, ii * VQ_T:(ii + 1) * VQ_T],
                        kT[:, (s + ii) * P:(s + ii + 1) * P],
                        qT[:, j * VQ_T:(j + 1) * VQ_T],
                        start=True, stop=True)
                nc.scalar.activation(
                    attn[:, s * VQ_T:(s + 2) * VQ_T],
                    sc[:],
                    mybir.ActivationFunctionType.Exp, scale=SCALE)

            def emit_consume(j, c):
                if c == 0:
                    zp = z_ps.tile([P, VQ_T], F32, tag="z", name=f"z{j}")
                    z_tiles[j] = zp
                zp = z_tiles[j]
                for ii in range(CHUNK):
                    i = CHUNK * c + ii
                    a_sl = attn[:, i * VQ_T:(i + 1) * VQ_T]
                    nc.tensor.matmul(
                        zp[:], y_sb[:, i, :], a_sl,
                        start=(i == 0), stop=(i == N_VK_T - 1))
                # VectorE partial-sum accumulation (both tiles in one op)
                ch_sl = attn[:, CHUNK * c * VQ_T:CHUNK * (c + 1) * VQ_T]
                if c == 0:
                    nc.vector.tensor_copy(acc[:], ch_sl)
                else:
                    nc.vector.tensor_tensor(
                        out=acc[:], in0=acc[:], in1=ch_sl,
                        op=mybir.AluOpType.add)
                if c == N_CH - 1:
                    jsl = slice(j * VQ_T, (j + 1) * VQ_T)
                    nc.vector.tensor_tensor(
                        out=acc2[:, jsl], in0=acc[:, 0:VQ_T],
                        in1=acc[:, VQ_T:], op=mybir.AluOpType.add)
                    nc.vector.tensor_copy(z_sb[:, jsl], zp[:])

            # staging schedule: all DMAs + x projections land up front so no
            # DVE cast in the flash region can block the in-order DVE queue
            # on a late transfer (that chains every exp to the prior chunk's
            # Vector add and halves throughput).
            pre = [lambda: dma_yT(0), lambda: dma_yT(1), dma_wqk, dma_x_all,
                   lambda: dma_y_raw(0), lambda: dma_y_raw(1),
                   lambda: dma_y_raw(2), lambda: dma_y_raw(3), dma_wv,
                   warmup,
                   lambda: proj_y(0), lambda: proj_y(1),
                   lambda: proj_x(0), lambda: proj_x(1),
                   lambda: proj_x(2), lambda: proj_x(3)]
            bg = [
                lambda: dma_yT(2),
                lambda: proj_y(2),
                lambda: dma_yT(3),
                lambda: proj_y(3),
                lambda: (dma_yT(4), dma_yT(5)),
                lambda: proj_y(4),
                lambda: proj_y(5),
                lambda: (dma_yT(6), dma_yT(7)),
                lambda: proj_y(6),
                lambda: proj_y(7),
            ]
            for t in pre:
                t()

            work = [(j, c) for j in range(N_VQ_T) for c in range(N_CH)]
            for n, (j, c) in enumerate(work):
                emit_scores_exp(j, c)
                if n < len(bg):
                    bg[n]()
                if n > 0:
                    emit_consume(*work[n - 1])
            emit_consume(*work[-1])

        # ---- tail: softmax sums + oT = Wv^T z, store ----
        with ExitStack() as fctx:
            f_ps = fctx.enter_context(
                tc.tile_pool(name="f_ps", bufs=2, space="PSUM"))
            s_ps = fctx.enter_context(
                tc.tile_pool(name="s_ps", bufs=2, space="PSUM"))
            for j in range(N_VQ_T):
                sl = slice(j * VQ_T, (j + 1) * VQ_T)
                sm = s_ps.tile([1, VQ_T], F32, tag="sm")
                nc.tensor.matmul(sm[:], ones_b[:], acc2[:, sl],
                                 start=True, stop=True)
                nc.scalar.copy(srow[:, sl], sm[:])
                o2 = f_ps.tile([P, VQ_T], F32, tag="o2")
                nc.tensor.matmul(o2[:], wv_sb[:], z_sb[:, sl],
                                 start=True, stop=True)
                nc.vector.tensor_copy(oT_sb[:, sl], o2[:])
                nc.sync.dma_start(oT_d[:, sl], oT_sb[:, sl])
            nc.sync.dma_start(sums_d[:], srow[:])

    _split_multi_waits(nc)
    return nc


_NC = None


def _get_nc():
    global _NC
    if _NC is None:
        _NC = _build()
    return _NC


def make_in_maps(x, y, Wq, Wk, Wv):
    bf = ml_dtypes.bfloat16
    x = np.ascontiguousarray(x, dtype=np.float32)
    y = np.ascontiguousarray(y, dtype=np.float32)
    wqk = np.ascontiguousarray(
        np.concatenate([Wq, Wk], axis=1), dtype=np.float32).astype(bf)
    wv = np.ascontiguousarray(Wv, dtype=np.float32)
    in_maps = []
    for core in range(8):
        n, half = core // 2, core % 2
        yb = y[n].astype(bf)
        # pre-tiled, partition-major: [p, t, c] for vk row = 128*t + p
        y_tiled = np.ascontiguousarray(
            yb.reshape(N_VK_T, P, C).transpose(1, 0, 2).reshape(P, -1))
        in_maps.append({
            "xT": np.ascontiguousarray(
                x[n, half * VQ_PER:(half + 1) * VQ_PER, :].T).astype(bf),
            "yT": np.ascontiguousarray(y[n].T).astype(bf),
            "y": y_tiled,
            "wqk": wqk, "wv": wv,
        })
    return in_maps


def finish(results):
    """Host-side epilogue: normalize + transpose per core shard."""
    out = np.empty((N, VQ, C), dtype=np.float32)
    for core in range(8):
        n, half = core // 2, core % 2
        r = results[core]
        out[n, half * VQ_PER:(half + 1) * VQ_PER, :] = (
            r["oT"] / r["sums"]).T
    return out


def kernel(x, y, Wq, Wk, Wv):
    nc = _get_nc()
    in_maps = make_in_maps(x, y, Wq, Wk, Wv)
    res = run_bass_kernel_spmd(nc, in_maps, list(range(8)))
    return finish(res.results)


# revision 20
# speedup vs baseline: 1.0038x; 1.0014x over previous
"""CrossAttention kernel for 8 trn2 NeuronCores (v6).

Reference:
  q = x @ Wq          [n, vq, h]
  k = y @ Wk          [n, vk, h]
  v = y @ Wv          [n, vk, c]
  out = softmax(q k^T / sqrt(h)) @ v        [n, vq, c]
with N=4, VQ=VK=4096, C=128, H=64, fp32.

Sharding: 8 cores = 4 batches x 2 query halves.

v6 (v1 154us, v2 135us, v3 103us, v4 112us, v5 130us):
  - fine-grained 512-col staging: tiny DMA->proj->cast chains so the first
    exp fires as soon as ~128KB have landed; critical descriptors first,
    staging spread across the two hwdge queues (sync: yT, scalar: wqk/xT/y).
  - bf16 everywhere on the PE streams (q/k/y/attn); exp input stays fp32
    PSUM, z accumulation fp32.
  - per chunk (2 vk tiles): PE = 2 score + 2 z streams (~1030ns), ScalarE =
    one exp [128,1024] (~1150ns, the bottleneck), VectorE = bf16 partial
    sums. Per-j folded sums parked in SBUF; all softmax-sum matmuls +
    normalization constants handled in the tail / on host.
  - PSUM: scores pool 3x2 banks (also used by proj bounces), z 2 banks.
    PE warm-up matmuls hide the HAM ramp during the first DMAs.
"""

import sys

sys.path.insert(0, "/opt/trn_rl_repo")

from contextlib import ExitStack

import ml_dtypes
import numpy as np

import concourse.bass as bass
import concourse.tile as tile
from concourse import mybir
from concourse.bass_utils import run_bass_kernel_spmd

F32 = mybir.dt.float32
F32R = mybir.dt.float32r
BF16 = mybir.dt.bfloat16
P = 128

N, VQ, VK, C, H = 4, 4096, 4096, 128, 64
VQ_PER = VQ // 2          # 2048 queries per core
SCALE = float(H) ** -0.5

VQ_T = 512                # vq tile (psum free dim)
N_VQ_T = VQ_PER // VQ_T   # 4
N_VK_T = VK // P          # 32 vk tiles of 128
CHUNK = 2                 # vk tiles per exp chunk
N_CH = N_VK_T // CHUNK    # 16 chunks per vq tile
ST = 512                  # staging granularity (columns)


def _split_multi_waits(nc):
    """walrus in this env supports one sync-wait per instruction; hoist
    extras onto same-engine NoOps inserted just before."""
    for fn in nc.m.functions:
        for bb in fn.blocks:
            out = []
            for inst in bb.instructions:
                si = inst.sync_info
                waits = list(si.on_wait) if si and si.on_wait else []
                if len(waits) > 1:
                    for w in waits[:-1]:
                        out.append(mybir.InstNoOp(
                            name=nc.get_next_instruction_name(),
                            engine=inst.engine,
                            ins=[], outs=[],
                            sync_info=mybir.SyncInfo(on_wait=[w], on_update=[]),
                        ))
                    inst.sync_info = mybir.SyncInfo(
                        on_wait=[waits[-1]],
                        on_update=list(si.on_update) if si.on_update else [],
                    )
                out.append(inst)
            bb.instructions = out


def _build():
    nc = bass.Bass()
    xT_d = nc.declare_dram_parameter("xT", [C, VQ_PER], BF16, isOutput=False)
    yT_d = nc.declare_dram_parameter("yT", [C, VK], BF16, isOutput=False)
    y_d = nc.declare_dram_parameter("y", [P, N_VK_T * P], BF16, isOutput=False)
    wqk_d = nc.declare_dram_parameter("wqk", [C, 2 * H], BF16, isOutput=False)
    wv_d = nc.declare_dram_parameter("wv", [C, C], F32R, isOutput=False)
    oT_d = nc.declare_dram_parameter("oT", [C, VQ_PER], F32, isOutput=True)
    sums_d = nc.declare_dram_parameter("sums", [1, VQ_PER], F32, isOutput=True)

    with tile.TileContext(nc) as tc, ExitStack() as ctx:
        const = ctx.enter_context(tc.tile_pool(name="const", bufs=1))
        persist = ctx.enter_context(tc.tile_pool(name="persist", bufs=1))

        # ---- constants (no DMA deps) ----
        wqk_sb = const.tile([P, 2 * H], BF16)
        wv_sb = const.tile([P, C], F32R)
        ones_f = const.tile([P, 1], F32)
        nc.vector.memset(ones_f[:], 1.0)
        ones_b = const.tile([P, 1], BF16)
        nc.vector.tensor_copy(ones_b[:], ones_f[:])
        junk_b = const.tile([P, VQ_T], BF16)
        nc.vector.memset(junk_b[:], 0.0)

        # ---- persistent tensors ----
        qT = persist.tile([64, VQ_PER], BF16)
        kT = persist.tile([64, VK], BF16)
        y_sb = persist.tile([P, N_VK_T, P], BF16)     # y tiles [vk, c]
        attn = persist.tile([P, N_VK_T * VQ_T], BF16)  # per vq tile, rotating
        acc = persist.tile([P, CHUNK * VQ_T], BF16)   # DVE partial sums
        acc2 = persist.tile([P, VQ_PER], BF16)        # folded sums, per j
        z_sb = persist.tile([P, VQ_PER], F32R)        # z = y^T attn  [c, vq]
        oT_sb = persist.tile([P, VQ_PER], F32)        # Wv^T z        [c, vq]
        srow = persist.tile([1, VQ_PER], F32)         # softmax sums

        with ExitStack() as mctx:
            # sc pool also hosts the projection bounces (smaller tiles)
            sc_ps = mctx.enter_context(
                tc.tile_pool(name="sc_ps", bufs=3, space="PSUM"))
            z_ps = mctx.enter_context(
                tc.tile_pool(name="z_ps", bufs=2, space="PSUM"))

            stage = mctx.enter_context(tc.tile_pool(name="stage", bufs=1))
            qx_stage = stage.tile([P, VQ_PER], BF16)
            ky_stage = stage.tile([P, VK], BF16)

            # ---- staging: DMA + projections (emitted interleaved below) --
            def dma_wqk():
                nc.scalar.dma_start(wqk_sb[:], wqk_d[:])

            def dma_wv():
                nc.scalar.dma_start(wv_sb[:], wv_d[:])

            def dma_x_all():  # xT in one contiguous DMA (512KB bf16)
                nc.sync.dma_start(qx_stage[:], xT_d[:])

            def dma_yT(s):  # s 0..7, 512 vk cols of yT
                nc.sync.dma_start(
                    ky_stage[:, s * ST:(s + 1) * ST],
                    yT_d[:, s * ST:(s + 1) * ST])

            def dma_y_raw(q):  # quarter q 0..3: 8 vk tiles, partition-major
                nc.scalar.dma_start(
                    y_sb[:, 8 * q:8 * (q + 1), :].rearrange("p t c -> p (t c)"),
                    y_d[:, 8 * q * P:8 * (q + 1) * P])

            def proj_x(s):
                sl = slice(s * ST, (s + 1) * ST)
                ps = sc_ps.tile([64, ST], F32, tag="sc", name=f"pjx{s}")
                nc.tensor.matmul(ps[:], wqk_sb[:, 0:H], qx_stage[:, sl],
                                 start=True, stop=True)
                nc.vector.tensor_copy(qT[:, sl], ps[:])

            def proj_y(s):
                sl = slice(s * ST, (s + 1) * ST)
                ps = sc_ps.tile([64, ST], F32, tag="sc", name=f"pjy{s}")
                nc.tensor.matmul(ps[:], wqk_sb[:, H:], ky_stage[:, sl],
                                 start=True, stop=True)
                nc.vector.tensor_copy(kT[:, sl], ps[:])

            def warmup():
                # keep the PE HAM-warm while the first DMAs land
                wu = z_ps.tile([P, VQ_T], F32, tag="z", name="wu")
                for _ in range(14):
                    nc.tensor.matmul(wu[0:1, :], ones_b[:], junk_b[:],
                                     start=True, stop=True)

            # ---- flash loop ----
            z_tiles = [None] * N_VQ_T

            def emit_scores_exp(j, c):
                sc = sc_ps.tile([P, CHUNK * VQ_T], F32, tag="sc")
                s = CHUNK * c
                for ii in range(CHUNK):
                    nc.tensor.matmul(
                        sc[:, ii * VQ_T:(ii + 1) * VQ_T],
                        kT[:, (s + ii) * P:(s + ii + 1) * P],
                        qT[:, j * VQ_T:(j + 1) * VQ_T],
                        start=True, stop=True)
                nc.scalar.activation(
                    attn[:, s * VQ_T:(s + 2) * VQ_T],
                    sc[:],
                    mybir.ActivationFunctionType.Exp, scale=SCALE)

            def emit_consume(j, c):
                if c == 0:
                    zp = z_ps.tile([P, VQ_T], F32, tag="z", name=f"z{j}")
                    z_tiles[j] = zp
                zp = z_tiles[j]
                for ii in range(CHUNK):
                    i = CHUNK * c + ii
                    a_sl = attn[:, i * VQ_T:(i + 1) * VQ_T]
                    nc.tensor.matmul(
                        zp[:], y_sb[:, i, :], a_sl,
                        start=(i == 0), stop=(i == N_VK_T - 1))
                # VectorE partial-sum accumulation (both tiles in one op)
                ch_sl = attn[:, CHUNK * c * VQ_T:CHUNK * (c + 1) * VQ_T]
                if c == 0:
                    nc.vector.tensor_copy(acc[:], ch_sl)
                else:
                    nc.vector.tensor_tensor(
                        out=acc[:], in0=acc[:], in1=ch_sl,
                        op=mybir.AluOpType.add)
                if c == N_CH - 1:
                    jsl = slice(j * VQ_T, (j + 1) * VQ_T)
                    nc.vector.tensor_tensor(
                        out=acc2[:, jsl], in0=acc[:, 0:VQ_T],
                        in1=acc[:, VQ_T:], op=mybir.AluOpType.add)
                    nc.vector.tensor_copy(z_sb[:, jsl], zp[:])

            # staging schedule: all DMAs + x projections land up front so no
            # DVE cast in the flash region can block the in-order DVE queue
            # on a late transfer (that chains every exp to the prior chunk's
            # Vector add and halves throughput).
            pre = [lambda: dma_yT(0), lambda: dma_yT(1), dma_wqk, dma_x_all,
                   lambda: dma_y_raw(0), lambda: dma_y_raw(1),
                   lambda: dma_y_raw(2), lambda: dma_y_raw(3), dma_wv,
                   lambda: proj_y(0), lambda: proj_y(1),
                   lambda: proj_x(0), lambda: proj_x(1),
                   lambda: proj_x(2), lambda: proj_x(3)]
            bg = [
                lambda: dma_yT(2),
                lambda: proj_y(2),
                lambda: dma_yT(3),
                lambda: proj_y(3),
                lambda: (dma_yT(4), dma_yT(5)),
                lambda: proj_y(4),
                lambda: proj_y(5),
                lambda: (dma_yT(6), dma_yT(7)),
                lambda: proj_y(6),
                lambda: proj_y(7),
            ]
            for t in pre:
                t()

            work = [(j, c) for j in range(N_VQ_T) for c in range(N_CH)]
            for n, (j, c) in enumerate(work):
                emit_scores_exp(j, c)
                if n < len(bg):
                    bg[n]()
                if n > 0:
                    emit_consume(*work[n - 1])
            emit_consume(*work[-1])

        # ---- tail: softmax sums + oT = Wv^T z, store ----
        with ExitStack() as fctx:
            f_ps = fctx.enter_context(
                tc.tile_pool(name="f_ps", bufs=2, space="PSUM"))
            s_ps = fctx.enter_context(
                tc.tile_pool(name="s_ps", bufs=2, space="PSUM"))
            for j in range(N_VQ_T):
                sl = slice(j * VQ_T, (j + 1) * VQ_T)
                sm = s_ps.tile([1, VQ_T], F32, tag="sm")
                nc.tensor.matmul(sm[:], ones_b[:], acc2[:, sl],
                                 start=True, stop=True)
                nc.scalar.copy(srow[:, sl], sm[:])
                o2 = f_ps.tile([P, VQ_T], F32, tag="o2")
                nc.tensor.matmul(o2[:], wv_sb[:], z_sb[:, sl],
                                 start=True, stop=True)
                nc.vector.tensor_copy(oT_sb[:, sl], o2[:])
                nc.sync.dma_start(oT_d[:, sl], oT_sb[:, sl])
            nc.sync.dma_start(sums_d[:], srow[:])

    _split_multi_waits(nc)
    return nc


_NC = None


def _get_nc():
    global _NC
    if _NC is None:
        _NC = _build()
    return _NC


def make_in_maps(x, y, Wq, Wk, Wv):
    bf = ml_dtypes.bfloat16
    x = np.ascontiguousarray(x, dtype=np.float32)
    y = np.ascontiguousarray(y, dtype=np.float32)
    wqk = np.ascontiguousarray(
        np.concatenate([Wq, Wk], axis=1), dtype=np.float32).astype(bf)
    wv = np.ascontiguousarray(Wv, dtype=np.float32)
    in_maps = []
    for core in range(8):
        n, half = core // 2, core % 2
        yb = y[n].astype(bf)
        # pre-tiled, partition-major: [p, t, c] for vk row = 128*t + p
        y_tiled = np.ascontiguousarray(
            yb.reshape(N_VK_T, P, C).transpose(1, 0, 2).reshape(P, -1))
        in_maps.append({
            "xT": np.ascontiguousarray(
                x[n, half * VQ_PER:(half + 1) * VQ_PER, :].T).astype(bf),
            "yT": np.ascontiguousarray(y[n].T).astype(bf),
            "y": y_tiled,
            "wqk": wqk, "wv": wv,
        })
    return in_maps


def finish(results):
    """Host-side epilogue: normalize + transpose per core shard."""
    out = np.empty((N, VQ, C), dtype=np.float32)
    for core in range(8):
        n, half = core // 2, core % 2
        r = results[core]
        out[n, half * VQ_PER:(half + 1) * VQ_PER, :] = (
            r["oT"] / r["sums"]).T
    return out


def kernel(x, y, Wq, Wk, Wv):
    nc = _get_nc()
    in_maps = make_in_maps(x, y, Wq, Wk, Wv)
    res = run_bass_kernel_spmd(nc, in_maps, list(range(8)))
    return finish(res.results)


# revision 21
# speedup vs baseline: 1.5122x; 1.5064x over previous
"""CrossAttention kernel for 8 trn2 NeuronCores (v6).

Reference:
  q = x @ Wq          [n, vq, h]
  k = y @ Wk          [n, vk, h]
  v = y @ Wv          [n, vk, c]
  out = softmax(q k^T / sqrt(h)) @ v        [n, vq, c]
with N=4, VQ=VK=4096, C=128, H=64, fp32.

Sharding: 8 cores = 4 batches x 2 query halves.

v6 (v1 154us, v2 135us, v3 103us, v4 112us, v5 130us):
  - fine-grained 512-col staging: tiny DMA->proj->cast chains so the first
    exp fires as soon as ~128KB have landed; critical descriptors first,
    staging spread across the two hwdge queues (sync: yT, scalar: wqk/xT/y).
  - bf16 everywhere on the PE streams (q/k/y/attn); exp input stays fp32
    PSUM, z accumulation fp32.
  - per chunk (2 vk tiles): PE = 2 score + 2 z streams (~1030ns), ScalarE =
    one exp [128,1024] (~1150ns, the bottleneck), VectorE = bf16 partial
    sums. Per-j folded sums parked in SBUF; all softmax-sum matmuls +
    normalization constants handled in the tail / on host.
  - PSUM: scores pool 3x2 banks (also used by proj bounces), z 2 banks.
    PE warm-up matmuls hide the HAM ramp during the first DMAs.
"""

import sys

sys.path.insert(0, "/opt/trn_rl_repo")

from contextlib import ExitStack

import ml_dtypes
import numpy as np

import concourse.bass as bass
import concourse.tile as tile
from concourse import mybir
from concourse.bass_utils import run_bass_kernel_spmd

F32 = mybir.dt.float32
F32R = mybir.dt.float32r
BF16 = mybir.dt.bfloat16
P = 128

N, VQ, VK, C, H = 4, 4096, 4096, 128, 64
VQ_PER = VQ // 2          # 2048 queries per core
SCALE = float(H) ** -0.5

VQ_T = 512                # vq tile (psum free dim)
N_VQ_T = VQ_PER // VQ_T   # 4
N_VK_T = VK // P          # 32 vk tiles of 128
CHUNK = 2                 # vk tiles per exp chunk
N_CH = N_VK_T // CHUNK    # 16 chunks per vq tile
ST = 512                  # staging granularity (columns)


def _split_multi_waits(nc):
    """walrus in this env supports one sync-wait per instruction; hoist
    extras onto same-engine NoOps inserted just before."""
    for fn in nc.m.functions:
        for bb in fn.blocks:
            out = []
            for inst in bb.instructions:
                si = inst.sync_info
                waits = list(si.on_wait) if si and si.on_wait else []
                if len(waits) > 1:
                    for w in waits[:-1]:
                        out.append(mybir.InstNoOp(
                            name=nc.get_next_instruction_name(),
                            engine=inst.engine,
                            ins=[], outs=[],
                            sync_info=mybir.SyncInfo(on_wait=[w], on_update=[]),
                        ))
                    inst.sync_info = mybir.SyncInfo(
                        on_wait=[waits[-1]],
                        on_update=list(si.on_update) if si.on_update else [],
                    )
                out.append(inst)
            bb.instructions = out


def _build():
    nc = bass.Bass()
    xT_d = nc.declare_dram_parameter("xT", [C, VQ_PER], BF16, isOutput=False)
    yT_d = nc.declare_dram_parameter("yT", [C, VK], BF16, isOutput=False)
    y_d = nc.declare_dram_parameter("y", [P, N_VK_T * P], BF16, isOutput=False)
    wqk_d = nc.declare_dram_parameter("wqk", [C, 2 * H], BF16, isOutput=False)
    wv_d = nc.declare_dram_parameter("wv", [C, C], F32R, isOutput=False)
    oT_d = nc.declare_dram_parameter("oT", [C, VQ_PER], F32, isOutput=True)
    sums_d = nc.declare_dram_parameter("sums", [1, VQ_PER], F32, isOutput=True)

    with tile.TileContext(nc) as tc, ExitStack() as ctx:
        const = ctx.enter_context(tc.tile_pool(name="const", bufs=1))
        persist = ctx.enter_context(tc.tile_pool(name="persist", bufs=1))

        # ---- constants (no DMA deps) ----
        wqk_sb = const.tile([P, 2 * H], BF16)
        wv_sb = const.tile([P, C], F32R)
        ones_f = const.tile([P, 1], F32)
        nc.vector.memset(ones_f[:], 1.0)
        ones_b = const.tile([P, 1], BF16)
        nc.vector.tensor_copy(ones_b[:], ones_f[:])
        junk_b = const.tile([P, VQ_T], BF16)
        nc.vector.memset(junk_b[:], 0.0)

        # ---- persistent tensors ----
        qT = persist.tile([64, VQ_PER], BF16)
        kT = persist.tile([64, VK], BF16)
        y_sb = persist.tile([P, N_VK_T, P], BF16)     # y tiles [vk, c]
        attn = persist.tile([P, N_VK_T * VQ_T], BF16)  # per vq tile, rotating
        acc = persist.tile([P, CHUNK * VQ_T], BF16)   # DVE partial sums
        acc2 = persist.tile([P, VQ_PER], BF16)        # folded sums, per j
        z_sb = persist.tile([P, VQ_PER], F32R)        # z = y^T attn  [c, vq]
        oT_sb = persist.tile([P, VQ_PER], F32)        # Wv^T z        [c, vq]
        srow = persist.tile([1, VQ_PER], F32)         # softmax sums

        with ExitStack() as mctx:
            # sc pool also hosts the projection bounces (smaller tiles)
            sc_ps = mctx.enter_context(
                tc.tile_pool(name="sc_ps", bufs=3, space="PSUM"))
            z_ps = mctx.enter_context(
                tc.tile_pool(name="z_ps", bufs=1, space="PSUM"))

            stage = mctx.enter_context(tc.tile_pool(name="stage", bufs=1))
            qx_stage = stage.tile([P, VQ_PER], BF16)
            ky_stage = stage.tile([P, VK], BF16)

            # ---- staging: DMA + projections (emitted interleaved below) --
            def dma_wqk():
                nc.scalar.dma_start(wqk_sb[:], wqk_d[:])

            def dma_wv():
                nc.scalar.dma_start(wv_sb[:], wv_d[:])

            def dma_x_all():  # xT in one contiguous DMA (512KB bf16)
                nc.sync.dma_start(qx_stage[:], xT_d[:])

            def dma_yT(s):  # s 0..7, 512 vk cols of yT
                nc.sync.dma_start(
                    ky_stage[:, s * ST:(s + 1) * ST],
                    yT_d[:, s * ST:(s + 1) * ST])

            def dma_y_raw(q):  # quarter q 0..3: 8 vk tiles, partition-major
                nc.scalar.dma_start(
                    y_sb[:, 8 * q:8 * (q + 1), :].rearrange("p t c -> p (t c)"),
                    y_d[:, 8 * q * P:8 * (q + 1) * P])

            def proj_x(s):
                sl = slice(s * ST, (s + 1) * ST)
                ps = sc_ps.tile([64, ST], F32, tag="sc", name=f"pjx{s}")
                nc.tensor.matmul(ps[:], wqk_sb[:, 0:H], qx_stage[:, sl],
                                 start=True, stop=True)
                nc.vector.tensor_copy(qT[:, sl], ps[:])

            def proj_y(s):
                sl = slice(s * ST, (s + 1) * ST)
                ps = sc_ps.tile([64, ST], F32, tag="sc", name=f"pjy{s}")
                nc.tensor.matmul(ps[:], wqk_sb[:, H:], ky_stage[:, sl],
                                 start=True, stop=True)
                nc.vector.tensor_copy(kT[:, sl], ps[:])

            def warmup():
                # keep the PE HAM-warm while the first DMAs land
                wu = z_ps.tile([P, VQ_T], F32, tag="z", name="wu")
                for _ in range(14):
                    nc.tensor.matmul(wu[0:1, :], ones_b[:], junk_b[:],
                                     start=True, stop=True)

            # ---- flash loop ----
            z_tiles = [None] * N_VQ_T

            def emit_scores_exp(j, c):
                sc = sc_ps.tile([P, CHUNK * VQ_T], F32, tag="sc")
                s = CHUNK * c
                for ii in range(CHUNK):
                    nc.tensor.matmul(
                        sc[:, ii * VQ_T:(ii + 1) * VQ_T],
                        kT[:, (s + ii) * P:(s + ii + 1) * P],
                        qT[:, j * VQ_T:(j + 1) * VQ_T],
                        start=True, stop=True)
                nc.scalar.activation(
                    attn[:, s * VQ_T:(s + 2) * VQ_T],
                    sc[:],
                    mybir.ActivationFunctionType.Exp, scale=SCALE)

            def emit_consume(j, c):
                if c == 0:
                    zp = z_ps.tile([P, VQ_T], F32, tag="z", name=f"z{j}")
                    z_tiles[j] = zp
                zp = z_tiles[j]
                for ii in range(CHUNK):
                    i = CHUNK * c + ii
                    a_sl = attn[:, i * VQ_T:(i + 1) * VQ_T]
                    nc.tensor.matmul(
                        zp[:], y_sb[:, i, :], a_sl,
                        start=(i == 0), stop=(i == N_VK_T - 1))
                # VectorE partial-sum accumulation (both tiles in one op)
                ch_sl = attn[:, CHUNK * c * VQ_T:CHUNK * (c + 1) * VQ_T]
                if c == 0:
                    nc.vector.tensor_copy(acc[:], ch_sl)
                else:
                    nc.vector.tensor_tensor(
                        out=acc[:], in0=acc[:], in1=ch_sl,
                        op=mybir.AluOpType.add)
                if c == N_CH - 1:
                    jsl = slice(j * VQ_T, (j + 1) * VQ_T)
                    nc.vector.tensor_tensor(
                        out=acc2[:, jsl], in0=acc[:, 0:VQ_T],
                        in1=acc[:, VQ_T:], op=mybir.AluOpType.add)
                    nc.vector.tensor_copy(z_sb[:, jsl], zp[:])

            # staging schedule: all DMAs + x projections land up front so no
            # DVE cast in the flash region can block the in-order DVE queue
            # on a late transfer (that chains every exp to the prior chunk's
            # Vector add and halves throughput).
            pre = [lambda: dma_yT(0), lambda: dma_yT(1), dma_wqk, dma_x_all,
                   lambda: dma_y_raw(0), lambda: dma_y_raw(1),
                   lambda: dma_y_raw(2), lambda: dma_y_raw(3), dma_wv,
                   lambda: proj_y(0), lambda: proj_y(1),
                   lambda: proj_x(0), lambda: proj_x(1),
                   lambda: proj_x(2), lambda: proj_x(3)]
            bg = [
                lambda: dma_yT(2),
                lambda: proj_y(2),
                lambda: dma_yT(3),
                lambda: proj_y(3),
                lambda: (dma_yT(4), dma_yT(5)),
                lambda: proj_y(4),
                lambda: proj_y(5),
                lambda: (dma_yT(6), dma_yT(7)),
                lambda: proj_y(6),
                lambda: proj_y(7),
            ]
            for t in pre:
                t()

            work = [(j, c) for j in range(N_VQ_T) for c in range(N_CH)]
            for n, (j, c) in enumerate(work):
                emit_scores_exp(j, c)
                if n < len(bg):
                    bg[n]()
                if n > 0:
                    emit_consume(*work[n - 1])
            emit_consume(*work[-1])

        # ---- tail: softmax sums + oT = Wv^T z, store ----
        with ExitStack() as fctx:
            f_ps = fctx.enter_context(
                tc.tile_pool(name="f_ps", bufs=2, space="PSUM"))
            s_ps = fctx.enter_context(
                tc.tile_pool(name="s_ps", bufs=2, space="PSUM"))
            for j in range(N_VQ_T):
                sl = slice(j * VQ_T, (j + 1) * VQ_T)
                sm = s_ps.tile([1, VQ_T], F32, tag="sm")
                nc.tensor.matmul(sm[:], ones_b[:], acc2[:, sl],
                                 start=True, stop=True)
                nc.scalar.copy(srow[:, sl], sm[:])
                o2 = f_ps.tile([P, VQ_T], F32, tag="o2")
                nc.tensor.matmul(o2[:], wv_sb[:], z_sb[:, sl],
                                 start=True, stop=True)
                nc.vector.tensor_copy(oT_sb[:, sl], o2[:])
                nc.sync.dma_start(oT_d[:, sl], oT_sb[:, sl])
            nc.sync.dma_start(sums_d[:], srow[:])

    _split_multi_waits(nc)
    return nc


_NC = None


def _get_nc():
    global _NC
    if _NC is None:
        _NC = _build()
    return _NC


def make_in_maps(x, y, Wq, Wk, Wv):
    bf = ml_dtypes.bfloat16
    x = np.ascontiguousarray(x, dtype=np.float32)
    y = np.ascontiguousarray(y, dtype=np.float32)
    wqk = np.ascontiguousarray(
        np.concatenate([Wq, Wk], axis=1), dtype=np.float32).astype(bf)
    wv = np.ascontiguousarray(Wv, dtype=np.float32)
    in_maps = []
    for core in range(8):
        n, half = core // 2, core % 2
        yb = y[n].astype(bf)
        # pre-tiled, partition-major: [p, t, c] for vk row = 128*t + p
        y_tiled = np.ascontiguousarray(
            yb.reshape(N_VK_T, P, C).transpose(1, 0, 2).reshape(P, -1))
        in_maps.append({
            "xT": np.ascontiguousarray(
                x[n, half * VQ_PER:(half + 1) * VQ_PER, :].T).astype(bf),
            "yT": np.ascontiguousarray(y[n].T).astype(bf),
            "y": y_tiled,
            "wqk": wqk, "wv": wv,
        })
    return in_maps


def finish(results):
    """Host-side epilogue: normalize + transpose per core shard."""
    out = np.empty((N, VQ, C), dtype=np.float32)
    for core in range(8):
        n, half = core // 2, core % 2
        r = results[core]
        out[n, half * VQ_PER:(half + 1) * VQ_PER, :] = (
            r["oT"] / r["sums"]).T
    return out


def kernel(x, y, Wq, Wk, Wv):
    nc = _get_nc()
    in_maps = make_in_maps(x, y, Wq, Wk, Wv)
    res = run_bass_kernel_spmd(nc, in_maps, list(range(8)))
    return finish(res.results)
